# revision 5
# baseline (speedup 1.0000x reference)
"""Trainium2 Bass kernel for nn_APLoss (8 NeuronCores, SPMD row-sharded).

The reference loss collapses to per-row quantities: with c_i = 1 - y_pred[i],
s_ij = relu(c_i + y_pred[j])^2,  R_i = sum_j s_ij,  Rpos_i = sum_{j<2048} s_ij:

  u_a = 0.01*u_all[i] + 0.99*R_i/n        u_p = 0.01*u_pos[i] + 0.99*Rpos_i/n
  loss = mean_i[ (u_p * R_i/n)/u_a^2 - (Rpos_i/n)/u_a ]

Each core owns 256 of the 2048 positive rows (2 partition blocks), holds all of
y_pred, computes its partial sum of the per-row terms; the host sums the 8
partial scalars (the unshard step for a row-sharded scalar mean).

Per-core device pipeline:
  * ScalarE: for the positive block j in [0, 2048): T = Relu(y_j + c_i)
    (per-partition bias), then Square with accum_out -> Rpos row sums.
  * VectorE: for j-chunks in [2048, 16384): B[j, r] = (f_r - (y_j+1) < 0) via
    one tensor_scalar per 128-j chunk (bf16 out - the 0/1 indicator is exact).
  * TensorE: (K,S1,S2) = W_chunk^T @ B accumulated in PSUM over chunks (bf16
    inputs, fp32 accumulate), where W = [1, y_j, y_j^2]; then
    R_tail = (c*K + 2*S1)*c + S2.
  * Small fp32 vector ops do the u-update / p / mean math; PE ones-matmul
    reduces over partitions; each core DMAs out one scalar.
"""

import numpy as np

try:
    import concourse.bass as bass  # noqa: F401
except ImportError:  # pragma: no cover
    import sys

    sys.path.insert(0, "/opt/trn_rl_repo")

N = 16384
P = 2048
NCORES = 8
RPC = P // NCORES  # 256 rows per core = 2 partition blocks
SA = 2048  # ScalarE covers j in [0, SA) = the positive block
K0 = SA // 128  # first DVE j-chunk (16)
ND = (N - SA) // 128  # number of DVE j-chunks (112)
GAMMA = 0.99

_NC_CACHE = {}


def _build_nc():
    import concourse.tile as tile
    from concourse import bacc, mybir

    f32 = mybir.dt.float32
    bf16 = mybir.dt.bfloat16
    Alu = mybir.AluOpType
    Act = mybir.ActivationFunctionType

    nc = bacc.Bacc("TRN2", target_bir_lowering=False, debug=False, num_devices=NCORES)

    # bf16 inputs feed the indicator/matmul path; f32 inputs feed the exact
    # ScalarE path and the final per-row math.
    y_bfrow = nc.dram_tensor("y_bfrow", [1, SA], bf16, kind="ExternalInput").ap()
    y_cols = nc.dram_tensor("y_cols", [128, 128], f32, kind="ExternalInput").ap()
    f_bfrow = nc.dram_tensor("f_bfrow", [1, RPC], bf16, kind="ExternalInput").ap()
    f_cols = nc.dram_tensor("f_cols", [128, 2], f32, kind="ExternalInput").ap()
    ua_cols = nc.dram_tensor("ua_cols", [128, 2], f32, kind="ExternalInput").ap()
    up_cols = nc.dram_tensor("up_cols", [128, 2], f32, kind="ExternalInput").ap()
    eye3 = nc.dram_tensor("eye3", [3, 3], f32, kind="ExternalInput").ap()
    ones_col = nc.dram_tensor("ones_col", [128, 1], f32, kind="ExternalInput").ap()
    out = nc.dram_tensor("out", [1, 1], f32, kind="ExternalOutput").ap()

    with tile.TileContext(nc) as tc:
        with (
            tc.tile_pool(name="const", bufs=1) as cpool,
            tc.tile_pool(name="bpool", bufs=6) as bpool,
            tc.tile_pool(name="scratch", bufs=2) as spool,
            tc.tile_pool(name="small", bufs=1) as mpool,
            tc.tile_pool(name="psum", bufs=1, space="PSUM") as ppool,
        ):
            # small fast HWDGE loads
            ycols = cpool.tile([128, 128], f32)
            nc.sync.dma_start(ycols[:], y_cols[:])
            fcols = cpool.tile([128, 2], f32)
            nc.sync.dma_start(fcols[:], f_cols[:])
            ua = cpool.tile([128, 2], f32)
            nc.sync.dma_start(ua[:], ua_cols[:])
            up = cpool.tile([128, 2], f32)
            nc.sync.dma_start(up[:], up_cols[:])
            eye3t = cpool.tile([3, 3], f32)
            nc.sync.dma_start(eye3t[:], eye3[:])
            onest = cpool.tile([128, 1], f32)
            nc.sync.dma_start(onest[:], ones_col[:])

            # broadcast of the per-core row slice (64KB bf16) - gates the
            # B-path, so it goes first on the gpsimd SWDGE queue.
            Fb = cpool.tile([128, RPC], bf16)
            nc.gpsimd.dma_start(Fb[:], f_bfrow[:].to_broadcast((128, RPC)))
            # broadcast of y (bf16, 512KB) for the ScalarE path, split across
            # two SWDGE queues so it overlaps with the B-path.
            Yb = cpool.tile([128, SA], bf16)
            nc.scalar.dma_start(
                Yb[:, 0 : SA // 2], y_bfrow[0:1, 0 : SA // 2].to_broadcast((128, SA // 2))
            )
            nc.gpsimd.dma_start(
                Yb[:, SA // 2 : SA],
                y_bfrow[0:1, SA // 2 : SA].to_broadcast((128, SA // 2)),
            )

            # y + 1 per-chunk bias columns (bf16 to match the tensor_scalar
            # input); c = 1 - f per-block bias columns (f32, exact path)
            y1 = cpool.tile([128, 128], f32)
            nc.vector.tensor_scalar_add(y1[:], ycols[:], 1.0)
            c_cols = cpool.tile([128, 2], f32)
            nc.scalar.activation(c_cols[:], fcols[:], Act.Identity, bias=1.0, scale=-1.0)

            # W[:, t, :] = [1, y, y^2] in bf16 for chunk k = K0 + t
            W = cpool.tile([128, ND, 3], bf16)
            nc.scalar.activation(W[:, :, 0], ycols[:, K0:128], Act.Copy, bias=1.0, scale=0.0)
            nc.vector.tensor_copy(W[:, :, 1], ycols[:, K0:128])
            nc.scalar.activation(W[:, :, 2], ycols[:, K0:128], Act.Square)

            # --- VectorE + TensorE path: j-chunks [K0, 128) ---
            psumQ = ppool.tile([3, RPC], f32)
            for t in range(ND):
                k = K0 + t
                Bt = bpool.tile([128, RPC], bf16)
                nc.vector.tensor_scalar(
                    Bt[:], Fb[:], y1[:, k : k + 1], 0.0, Alu.subtract, Alu.is_lt
                )
                nc.tensor.matmul(
                    psumQ[:], W[:, t, :], Bt[:], start=(t == 0), stop=(t == ND - 1)
                )

            # --- ScalarE path: positive block j in [0, 2048) ---
            accPos = mpool.tile([128, 2], f32)
            for b in range(2):
                cb = c_cols[:, b : b + 1]
                t1 = spool.tile([128, P], f32, tag="t1")
                nc.scalar.activation(t1[:], Yb[:, 0:P], Act.Relu, bias=cb)
                t2 = spool.tile([128, P], f32, tag="t2")
                nc.scalar.activation(
                    t2[:], t1[:], Act.Square, accum_out=accPos[:, b : b + 1]
                )

            # transpose (3, 256) -> per-row (128, 2, 3) via PE with identity
            sbQ = mpool.tile([3, RPC], f32)
            nc.scalar.copy(sbQ[:], psumQ[:])
            psumT = ppool.tile([128, 2, 3], f32)
            for h in range(2):
                nc.tensor.matmul(
                    psumT[:, h, :],
                    sbQ[:, h * 128 : (h + 1) * 128],
                    eye3t[:],
                    start=True,
                    stop=True,
                )
            Kq = psumT[:, :, 0]
            S1q = psumT[:, :, 1]
            S2q = psumT[:, :, 2]

            # R_tail = (c*K + 2*S1)*c + S2
            w1 = mpool.tile([128, 2], f32)
            nc.vector.tensor_tensor(w1[:], c_cols[:], Kq, Alu.mult)
            w2 = mpool.tile([128, 2], f32)
            nc.vector.scalar_tensor_tensor(w2[:], S1q, 2.0, w1[:], Alu.mult, Alu.add)
            w3 = mpool.tile([128, 2], f32)
            nc.vector.tensor_tensor(w3[:], w2[:], c_cols[:], Alu.mult)
            Rt = mpool.tile([128, 2], f32)
            nc.vector.tensor_tensor(Rt[:], w3[:], S2q, Alu.add)

            R = mpool.tile([128, 2], f32)
            nc.vector.tensor_tensor(R[:], accPos[:], Rt[:], Alu.add)

            # u updates, p, and the mean
            uas = mpool.tile([128, 2], f32)
            nc.vector.tensor_scalar_mul(uas[:], ua[:], 1.0 - GAMMA)
            ups = mpool.tile([128, 2], f32)
            nc.vector.tensor_scalar_mul(ups[:], up[:], 1.0 - GAMMA)
            uan = mpool.tile([128, 2], f32)
            nc.vector.scalar_tensor_tensor(uan[:], R[:], GAMMA / N, uas[:], Alu.mult, Alu.add)
            upn = mpool.tile([128, 2], f32)
            nc.vector.scalar_tensor_tensor(
                upn[:], accPos[:], GAMMA / N, ups[:], Alu.mult, Alu.add
            )
            inv = mpool.tile([128, 2], f32)
            nc.vector.reciprocal(inv[:], uan[:])
            g1 = mpool.tile([128, 2], f32)
            nc.vector.tensor_tensor(g1[:], upn[:], R[:], Alu.mult)
            g2 = mpool.tile([128, 2], f32)
            nc.vector.tensor_tensor(g2[:], g1[:], inv[:], Alu.mult)
            g3 = mpool.tile([128, 2], f32)
            nc.vector.tensor_tensor(g3[:], g2[:], accPos[:], Alu.subtract)
            g4 = mpool.tile([128, 2], f32)
            nc.vector.tensor_tensor(g4[:], g3[:], inv[:], Alu.mult)
            acc = mpool.tile([128, 1], f32)
            nc.vector.tensor_reduce(acc[:], g4[:], mybir.AxisListType.X, Alu.add)
            psumF = ppool.tile([1, 1], f32)
            nc.tensor.matmul(psumF[:], onest[:], acc[:], start=True, stop=True)
            outsb = mpool.tile([1, 1], f32)
            # fold the 1/(N*P) mean normalization into the final copy
            nc.scalar.mul(outsb[:], psumF[:], 1.0 / (float(N) * float(P)))
            nc.sync.dma_start(out[:], outsb[:])

    nc.compile()
    return nc


def get_nc():
    if "nc" not in _NC_CACHE:
        _NC_CACHE["nc"] = _build_nc()
    return _NC_CACHE["nc"]


def make_in_maps(y_pred, u_all, u_pos, index_s, n_pos):
    import ml_dtypes

    y = np.ascontiguousarray(np.asarray(y_pred, dtype=np.float32).reshape(N))
    u_all = np.asarray(u_all, dtype=np.float32).reshape(-1)
    u_pos = np.asarray(u_pos, dtype=np.float32).reshape(-1)
    idx = np.asarray(index_s).astype(np.int64).reshape(-1)[:P]
    ua_ps = u_all[idx]
    up_ps = u_pos[idx]
    f = y[:P]

    y_cols = np.ascontiguousarray(y.reshape(128, 128).T)  # [p, k] = y[k*128 + p]
    y_bfrow = y[:SA].reshape(1, SA).astype(ml_dtypes.bfloat16)
    eye3 = np.eye(3, dtype=np.float32)
    ones_col = np.ones((128, 1), dtype=np.float32)

    in_maps = []
    for c in range(NCORES):
        rows = slice(c * RPC, (c + 1) * RPC)
        in_maps.append(
            {
                "y_bfrow": y_bfrow,
                "y_cols": y_cols,
                "f_bfrow": np.ascontiguousarray(
                    f[rows].reshape(1, RPC).astype(ml_dtypes.bfloat16)
                ),
                "f_cols": np.ascontiguousarray(f[rows].reshape(2, 128).T),
                "ua_cols": np.ascontiguousarray(ua_ps[rows].reshape(2, 128).T),
                "up_cols": np.ascontiguousarray(up_ps[rows].reshape(2, 128).T),
                "eye3": eye3,
                "ones_col": ones_col,
            }
        )
    return in_maps


def kernel(**inputs):
    n_pos = int(np.asarray(inputs["n_pos"]))
    assert n_pos == P, f"kernel hardcodes n_pos={P}, got {n_pos}"
    in_maps = make_in_maps(
        inputs["y_pred"], inputs["u_all"], inputs["u_pos"], inputs["index_s"], n_pos
    )
    from concourse.bass_utils import run_bass_kernel_spmd

    nc = get_nc()
    res = run_bass_kernel_spmd(nc, in_maps, list(range(NCORES)))
    total = 0.0
    for r in res.results:
        total += float(r["out"][0, 0])
    return np.float32(total)


# revision 8
# speedup vs baseline: 1.1379x; 1.1379x over previous
"""Trainium2 Bass kernel for nn_APLoss (8 NeuronCores, SPMD row-sharded).

The reference loss collapses to per-row quantities: with c_i = 1 - y_pred[i],
s_ij = relu(c_i + y_pred[j])^2,  R_i = sum_j s_ij,  Rpos_i = sum_{j<2048} s_ij:

  u_a = 0.01*u_all[i] + 0.99*R_i/n        u_p = 0.01*u_pos[i] + 0.99*Rpos_i/n
  loss = mean_i[ (u_p * R_i/n)/u_a^2 - (Rpos_i/n)/u_a ]

Each core owns 256 of the 2048 positive rows (2 partition blocks), holds all of
y_pred, computes its partial sum of the per-row terms; the host sums the 8
partial scalars (the unshard step for a row-sharded scalar mean).

Per-core device pipeline (three compute engines on the pairwise work):
  * ScalarE, positive block j in [0, 2048): T = Relu(y_j + c_i) with
    per-partition bias, then Square with accum_out -> Rpos row sums (exact).
  * Indicator path for j in [2048, 16384): sum_j relu(x)^2 over the active set
    {x = c_i + y_j > 0} equals c^2*K + 2c*S1 + S2 with (K, S1, S2) = B^T W,
    W = [1, y, y^2]. B chunks ((128 j) x (256 rows)) are produced two ways:
      - ScalarE chunks: S = Sign(y_j + 1 - f_r) in {-1,+1}; B = (S+1)/2, the
        +1 correction folded in afterwards via column sums of W.
      - VectorE chunks: B = (f_r - (y_j+1) < 0) directly via tensor_scalar.
    TensorE accumulates W_chunk^T @ {S|B} into PSUM (bf16 in, fp32 acc).
  * Small fp32 vector ops do the u-update / p / mean math; PE ones-matmul
    reduces over partitions; each core DMAs out one scalar.
"""

import numpy as np

try:
    import concourse.bass as bass  # noqa: F401
except ImportError:  # pragma: no cover
    import sys

    sys.path.insert(0, "/opt/trn_rl_repo")

N = 16384
P = 2048
NCORES = 8
RPC = P // NCORES  # 256 rows per core = 2 partition blocks
SA = 2048  # ScalarE relu^2 covers j in [0, SA) = the positive block
K0 = SA // 128  # first indicator j-chunk (16)
ND = (N - SA) // 128  # number of indicator j-chunks (112)
NA = 34  # of which: chunks produced by ScalarE Sign
GAMMA = 0.99

_NC_CACHE = {}


def _build_nc():
    import concourse.tile as tile
    from concourse import bacc, mybir

    f32 = mybir.dt.float32
    bf16 = mybir.dt.bfloat16
    Alu = mybir.AluOpType
    Act = mybir.ActivationFunctionType

    nc = bacc.Bacc("TRN2", target_bir_lowering=False, debug=False, num_devices=NCORES)

    y_bfrow = nc.dram_tensor("y_bfrow", [1, SA], bf16, kind="ExternalInput").ap()
    y_cols = nc.dram_tensor("y_cols", [128, 128], f32, kind="ExternalInput").ap()
    f_bfrow = nc.dram_tensor("f_bfrow", [1, RPC], bf16, kind="ExternalInput").ap()
    f_cols = nc.dram_tensor("f_cols", [128, 2], f32, kind="ExternalInput").ap()
    ua_cols = nc.dram_tensor("ua_cols", [128, 2], f32, kind="ExternalInput").ap()
    up_cols = nc.dram_tensor("up_cols", [128, 2], f32, kind="ExternalInput").ap()
    eye3 = nc.dram_tensor("eye3", [3, 3], f32, kind="ExternalInput").ap()
    ones_col = nc.dram_tensor("ones_col", [128, 1], f32, kind="ExternalInput").ap()
    out = nc.dram_tensor("out", [1, 1], f32, kind="ExternalOutput").ap()

    with tile.TileContext(nc) as tc:
        with (
            tc.tile_pool(name="const", bufs=1) as cpool,
            tc.tile_pool(name="bpool", bufs=8) as bpool,
            tc.tile_pool(name="spool2", bufs=4) as apool,
            tc.tile_pool(name="scratch", bufs=2) as spool,
            tc.tile_pool(name="small", bufs=1) as mpool,
            tc.tile_pool(name="psum", bufs=1, space="PSUM") as ppool,
        ):
            # small fast HWDGE loads
            ycols = cpool.tile([128, 128], f32)
            nc.sync.dma_start(ycols[:], y_cols[:])
            fcols = cpool.tile([128, 2], f32)
            nc.sync.dma_start(fcols[:], f_cols[:])
            ua = cpool.tile([128, 2], f32)
            nc.sync.dma_start(ua[:], ua_cols[:])
            up = cpool.tile([128, 2], f32)
            nc.sync.dma_start(up[:], up_cols[:])
            eye3t = cpool.tile([3, 3], f32)
            nc.sync.dma_start(eye3t[:], eye3[:])
            onest = cpool.tile([128, 1], f32)
            nc.sync.dma_start(onest[:], ones_col[:])

            # broadcast of the per-core row slice (64KB bf16) gates all
            # indicator work - put it on the Vector queue (earliest consumer).
            Fb = cpool.tile([128, RPC], bf16)
            nc.scalar.dma_start(Fb[:], f_bfrow[:].to_broadcast((128, RPC)))
            # y broadcast (bf16, 512KB) for the ScalarE relu^2 path, on two
            # other SWDGE queues so it overlaps with the indicator path.
            Yb = cpool.tile([128, SA], bf16)
            nc.scalar.dma_start(
                Yb[:, 0 : SA // 2], y_bfrow[0:1, 0 : SA // 2].to_broadcast((128, SA // 2))
            )
            nc.gpsimd.dma_start(
                Yb[:, SA // 2 : SA],
                y_bfrow[0:1, SA // 2 : SA].to_broadcast((128, SA // 2)),
            )

            # y + 1 per-chunk bias columns; c = 1 - f per-block bias columns
            y1 = cpool.tile([128, 128], f32)
            nc.vector.tensor_scalar_add(y1[:], ycols[:], 1.0)
            c_cols = cpool.tile([128, 2], f32)
            nc.scalar.activation(c_cols[:], fcols[:], Act.Identity, bias=1.0, scale=-1.0)
            onesbf = cpool.tile([128, 1], bf16)
            nc.vector.tensor_copy(onesbf[:], onest[:])

            # W[:, t, :] = [1, y, y^2] in bf16 for chunk k = K0 + t
            W = cpool.tile([128, ND, 3], bf16)
            nc.scalar.activation(W[:, :, 0], ycols[:, K0:128], Act.Copy, bias=1.0, scale=0.0)
            nc.vector.tensor_copy(W[:, :, 1], ycols[:, K0:128])
            nc.scalar.activation(W[:, :, 2], ycols[:, K0:128], Act.Square)

            # --- indicator chunks: ScalarE Sign for t in [0, NA), VectorE
            # tensor_scalar for t in [NA, ND); PE matmuls interleaved in
            # rough production order so neither producer stalls on PSUM. ---
            psumS = ppool.tile([3, RPC], f32)
            psumQ = ppool.tile([3, RPC], f32)
            tA = 0
            tD = NA
            nDV = ND - NA
            for step in range(ND):
                # interleave: pick the stream whose progress fraction is lower
                doA = tA < NA and (tD >= ND or (tA * nDV <= (tD - NA) * NA))
                if doA:
                    t = tA
                    k = K0 + t
                    St = apool.tile([128, RPC], bf16, tag="sg")
                    nc.scalar.activation(
                        St[:], Fb[:], Act.Sign, bias=y1[:, k : k + 1], scale=-1.0
                    )
                    nc.tensor.matmul(
                        psumS[:], W[:, t, :], St[:], start=(t == 0), stop=(t == NA - 1)
                    )
                    tA += 1
                else:
                    t = tD
                    k = K0 + t
                    Bt = bpool.tile([128, RPC], bf16, tag="bt")
                    nc.vector.tensor_scalar(
                        Bt[:], Fb[:], y1[:, k : k + 1], 0.0, Alu.subtract, Alu.is_lt
                    )
                    nc.tensor.matmul(
                        psumQ[:], W[:, t, :], Bt[:], start=(t == NA), stop=(t == ND - 1)
                    )
                    tD += 1

            # column sums of W over the ScalarE chunks: [sum 1, sum y, sum y^2]
            psumW = ppool.tile([1, NA, 3], f32)
            nc.tensor.matmul(
                psumW[:], onesbf[:], W[:, 0:NA, :].rearrange("p t c -> p (t c)"),
                start=True, stop=True,
            )
            w1row = mpool.tile([1, 3], f32)
            nc.vector.tensor_reduce(
                w1row[:], psumW[:].rearrange("p t c -> p c t"), mybir.AxisListType.X, Alu.add
            )
            psumWT = ppool.tile([3, 1], f32)
            nc.tensor.matmul(psumWT[:], w1row[:], onest[0:1, :], start=True, stop=True)

            # --- ScalarE path: positive block j in [0, 2048) ---
            accPos = mpool.tile([128, 2], f32)
            for b in range(2):
                cb = c_cols[:, b : b + 1]
                t1 = spool.tile([128, P], f32, tag="t1")
                nc.scalar.activation(t1[:], Yb[:, 0:P], Act.Relu, bias=cb)
                t2 = spool.tile([128, P], f32, tag="t2")
                nc.scalar.activation(
                    t2[:], t1[:], Act.Square, accum_out=accPos[:, b : b + 1]
                )

            # combine (K,S1,S2) = psumQ + 0.5*psumS + 0.5*colsums(W_act)
            sbQ = mpool.tile([3, RPC], f32)
            nc.scalar.copy(sbQ[:], psumQ[:])
            sbC = mpool.tile([3, RPC], f32)
            nc.vector.scalar_tensor_tensor(
                sbC[:], psumS[:], 0.5, sbQ[:], Alu.mult, Alu.add
            )
            sbC2 = mpool.tile([3, RPC], f32)
            nc.vector.scalar_tensor_tensor(
                sbC2[:], psumWT[:].to_broadcast((3, RPC)), 0.5, sbC[:], Alu.mult, Alu.add
            )

            # transpose (3, 256) -> per-row (128, 2, 3) via PE with identity
            psumT = ppool.tile([128, 2, 3], f32)
            for h in range(2):
                nc.tensor.matmul(
                    psumT[:, h, :],
                    sbC2[:, h * 128 : (h + 1) * 128],
                    eye3t[:],
                    start=True,
                    stop=True,
                )
            Kq = psumT[:, :, 0]
            S1q = psumT[:, :, 1]
            S2q = psumT[:, :, 2]

            # R_tail = (c*K + 2*S1)*c + S2
            w1 = mpool.tile([128, 2], f32)
            nc.vector.tensor_tensor(w1[:], c_cols[:], Kq, Alu.mult)
            w2 = mpool.tile([128, 2], f32)
            nc.vector.scalar_tensor_tensor(w2[:], S1q, 2.0, w1[:], Alu.mult, Alu.add)
            w3 = mpool.tile([128, 2], f32)
            nc.vector.tensor_tensor(w3[:], w2[:], c_cols[:], Alu.mult)
            Rt = mpool.tile([128, 2], f32)
            nc.vector.tensor_tensor(Rt[:], w3[:], S2q, Alu.add)

            R = mpool.tile([128, 2], f32)
            nc.vector.tensor_tensor(R[:], accPos[:], Rt[:], Alu.add)

            # u updates, p, and the mean
            uas = mpool.tile([128, 2], f32)
            nc.vector.tensor_scalar_mul(uas[:], ua[:], 1.0 - GAMMA)
            ups = mpool.tile([128, 2], f32)
            nc.vector.tensor_scalar_mul(ups[:], up[:], 1.0 - GAMMA)
            uan = mpool.tile([128, 2], f32)
            nc.vector.scalar_tensor_tensor(uan[:], R[:], GAMMA / N, uas[:], Alu.mult, Alu.add)
            upn = mpool.tile([128, 2], f32)
            nc.vector.scalar_tensor_tensor(
                upn[:], accPos[:], GAMMA / N, ups[:], Alu.mult, Alu.add
            )
            inv = mpool.tile([128, 2], f32)
            nc.vector.reciprocal(inv[:], uan[:])
            g1 = mpool.tile([128, 2], f32)
            nc.vector.tensor_tensor(g1[:], upn[:], R[:], Alu.mult)
            g2 = mpool.tile([128, 2], f32)
            nc.vector.tensor_tensor(g2[:], g1[:], inv[:], Alu.mult)
            g3 = mpool.tile([128, 2], f32)
            nc.vector.tensor_tensor(g3[:], g2[:], accPos[:], Alu.subtract)
            g4 = mpool.tile([128, 2], f32)
            nc.vector.tensor_tensor(g4[:], g3[:], inv[:], Alu.mult)
            acc = mpool.tile([128, 1], f32)
            nc.vector.tensor_reduce(acc[:], g4[:], mybir.AxisListType.X, Alu.add)
            psumF = ppool.tile([1, 1], f32)
            nc.tensor.matmul(psumF[:], onest[:], acc[:], start=True, stop=True)
            outsb = mpool.tile([1, 1], f32)
            # fold the 1/(N*P) mean normalization into the final copy
            nc.scalar.mul(outsb[:], psumF[:], 1.0 / (float(N) * float(P)))
            nc.sync.dma_start(out[:], outsb[:])

    nc.compile()
    return nc


def get_nc():
    if "nc" not in _NC_CACHE:
        _NC_CACHE["nc"] = _build_nc()
    return _NC_CACHE["nc"]


def make_in_maps(y_pred, u_all, u_pos, index_s, n_pos):
    import ml_dtypes

    y = np.ascontiguousarray(np.asarray(y_pred, dtype=np.float32).reshape(N))
    u_all = np.asarray(u_all, dtype=np.float32).reshape(-1)
    u_pos = np.asarray(u_pos, dtype=np.float32).reshape(-1)
    idx = np.asarray(index_s).astype(np.int64).reshape(-1)[:P]
    ua_ps = u_all[idx]
    up_ps = u_pos[idx]
    f = y[:P]

    y_cols = np.ascontiguousarray(y.reshape(128, 128).T)  # [p, k] = y[k*128 + p]
    y_bfrow = y[:SA].reshape(1, SA).astype(ml_dtypes.bfloat16)
    eye3 = np.eye(3, dtype=np.float32)
    ones_col = np.ones((128, 1), dtype=np.float32)

    in_maps = []
    for c in range(NCORES):
        rows = slice(c * RPC, (c + 1) * RPC)
        in_maps.append(
            {
                "y_bfrow": y_bfrow,
                "y_cols": y_cols,
                "f_bfrow": np.ascontiguousarray(
                    f[rows].reshape(1, RPC).astype(ml_dtypes.bfloat16)
                ),
                "f_cols": np.ascontiguousarray(f[rows].reshape(2, 128).T),
                "ua_cols": np.ascontiguousarray(ua_ps[rows].reshape(2, 128).T),
                "up_cols": np.ascontiguousarray(up_ps[rows].reshape(2, 128).T),
                "eye3": eye3,
                "ones_col": ones_col,
            }
        )
    return in_maps


def kernel(**inputs):
    n_pos = int(np.asarray(inputs["n_pos"]))
    assert n_pos == P, f"kernel hardcodes n_pos={P}, got {n_pos}"
    in_maps = make_in_maps(
        inputs["y_pred"], inputs["u_all"], inputs["u_pos"], inputs["index_s"], n_pos
    )
    from concourse.bass_utils import run_bass_kernel_spmd

    nc = get_nc()
    res = run_bass_kernel_spmd(nc, in_maps, list(range(NCORES)))
    total = 0.0
    for r in res.results:
        total += float(r["out"][0, 0])
    return np.float32(total)


# revision 9
# speedup vs baseline: 1.1679x; 1.0264x over previous
"""Trainium2 Bass kernel for nn_APLoss (8 NeuronCores, SPMD row-sharded).

The reference loss collapses to per-row quantities: with c_i = 1 - y_pred[i],
s_ij = relu(c_i + y_pred[j])^2,  R_i = sum_j s_ij,  Rpos_i = sum_{j<2048} s_ij:

  u_a = 0.01*u_all[i] + 0.99*R_i/n        u_p = 0.01*u_pos[i] + 0.99*Rpos_i/n
  loss = mean_i[ (u_p * R_i/n)/u_a^2 - (Rpos_i/n)/u_a ]

Each core owns 256 of the 2048 positive rows (2 partition blocks), holds all of
y_pred, computes its partial sum of the per-row terms; the host sums the 8
partial scalars (the unshard step for a row-sharded scalar mean).

Per-core device pipeline:
  * ScalarE, j in [0, SA): T = Relu(y_j + c_i) with per-partition bias, then
    Square with accum_out -> row sums, split at j=2048 so Rpos falls out.
  * VectorE, j-chunks in [SA, 16384): B[j, r] = (f_r - (y_j+1) < 0) via one
    bf16 tensor_scalar per 128-j chunk (2 elem/cycle/lane).
  * TensorE: (K,S1,S2) = W_chunk^T @ B accumulated in PSUM over chunks (bf16
    in, fp32 acc), W = [1, y, y^2]; then R_tail = (c*K + 2*S1)*c + S2.
  * Small fp32 vector ops do the u-update / p / mean math; PE ones-matmul
    reduces over partitions; each core DMAs out one scalar.

Host-side prep is layout only: dtype casts, reshapes/transposes, the gather
u[index_s[:n_pos]], and replicating y/f slices across the 128 partitions.
"""

import numpy as np

try:
    import concourse.bass as bass  # noqa: F401
except ImportError:  # pragma: no cover
    import sys

    sys.path.insert(0, "/opt/trn_rl_repo")

N = 16384
P = 2048
NCORES = 8
RPC = P // NCORES  # 256 rows per core = 2 partition blocks
SA = 4096  # ScalarE covers j in [0, SA)
K0 = SA // 128  # first indicator j-chunk (32)
ND = (N - SA) // 128  # number of indicator j-chunks (96)
GAMMA = 0.99

_NC_CACHE = {}


def _build_nc():
    import concourse.tile as tile
    from concourse import bacc, mybir

    f32 = mybir.dt.float32
    bf16 = mybir.dt.bfloat16
    Alu = mybir.AluOpType
    Act = mybir.ActivationFunctionType

    nc = bacc.Bacc("TRN2", target_bir_lowering=False, debug=False, num_devices=NCORES)

    y_bc = nc.dram_tensor("y_bc", [128, SA], bf16, kind="ExternalInput").ap()
    f_bc = nc.dram_tensor("f_bc", [128, RPC], bf16, kind="ExternalInput").ap()
    y_cols = nc.dram_tensor("y_cols", [128, 128], f32, kind="ExternalInput").ap()
    f_cols = nc.dram_tensor("f_cols", [128, 2], f32, kind="ExternalInput").ap()
    ua_cols = nc.dram_tensor("ua_cols", [128, 2], f32, kind="ExternalInput").ap()
    up_cols = nc.dram_tensor("up_cols", [128, 2], f32, kind="ExternalInput").ap()
    eye3 = nc.dram_tensor("eye3", [3, 3], f32, kind="ExternalInput").ap()
    ones_col = nc.dram_tensor("ones_col", [128, 1], f32, kind="ExternalInput").ap()
    out = nc.dram_tensor("out", [1, 1], f32, kind="ExternalOutput").ap()

    with tile.TileContext(nc) as tc:
        with (
            tc.tile_pool(name="const", bufs=1) as cpool,
            tc.tile_pool(name="bpool", bufs=8) as bpool,
            tc.tile_pool(name="scratch", bufs=2) as spool,
            tc.tile_pool(name="small", bufs=1) as mpool,
            tc.tile_pool(name="psum", bufs=1, space="PSUM") as ppool,
        ):
            # the indicator path gate: f broadcast (64KB) on its own queue
            Fb = cpool.tile([128, RPC], bf16)
            nc.sync.dma_start(Fb[:], f_bc[:])
            # y broadcast split across HW queues; pos half first
            Yb = cpool.tile([128, SA], bf16)
            nq = 4
            for q in range(nq):
                s = q * (SA // nq)
                nc.sync.dma_start(Yb[:, s : s + SA // nq], y_bc[:, s : s + SA // nq])
            ycols = cpool.tile([128, 128], f32)
            nc.sync.dma_start(ycols[:], y_cols[:])
            fcols = cpool.tile([128, 2], f32)
            nc.sync.dma_start(fcols[:], f_cols[:])
            ua = cpool.tile([128, 2], f32)
            nc.sync.dma_start(ua[:], ua_cols[:])
            up = cpool.tile([128, 2], f32)
            nc.sync.dma_start(up[:], up_cols[:])
            eye3t = cpool.tile([3, 3], f32)
            nc.sync.dma_start(eye3t[:], eye3[:])
            onest = cpool.tile([128, 1], f32)
            nc.sync.dma_start(onest[:], ones_col[:])

            # y + 1 per-chunk bias columns; c = 1 - f per-block bias columns
            y1 = cpool.tile([128, 128], f32)
            nc.vector.tensor_scalar_add(y1[:], ycols[:], 1.0)
            c_cols = cpool.tile([128, 2], f32)
            nc.scalar.activation(c_cols[:], fcols[:], Act.Identity, bias=1.0, scale=-1.0)

            # W[:, t, :] = [1, y, y^2] in bf16 for chunk k = K0 + t
            W = cpool.tile([128, ND, 3], bf16)
            nc.scalar.activation(W[:, :, 0], ycols[:, K0:128], Act.Copy, bias=1.0, scale=0.0)
            nc.vector.tensor_copy(W[:, :, 1], ycols[:, K0:128])
            nc.scalar.activation(W[:, :, 2], ycols[:, K0:128], Act.Square)

            # --- VectorE + TensorE indicator path: j-chunks [K0, 128) ---
            psumQ = ppool.tile([3, RPC], f32)
            for t in range(ND):
                k = K0 + t
                Bt = bpool.tile([128, RPC], bf16, tag="bt")
                nc.vector.tensor_scalar(
                    Bt[:], Fb[:], y1[:, k : k + 1], 0.0, Alu.subtract, Alu.is_lt
                )
                nc.tensor.matmul(
                    psumQ[:], W[:, t, :], Bt[:], start=(t == 0), stop=(t == ND - 1)
                )

            # --- ScalarE path: j in [0, SA), split at the positive boundary ---
            accPos = mpool.tile([128, 2], f32)
            accRest = mpool.tile([128, 2], f32)
            for b in range(2):
                cb = c_cols[:, b : b + 1]
                t1 = spool.tile([128, P], f32, tag="t1")
                nc.scalar.activation(t1[:], Yb[:, 0:P], Act.Relu, bias=cb)
                t2 = spool.tile([128, P], f32, tag="t2")
                nc.scalar.activation(
                    t2[:], t1[:], Act.Square, accum_out=accPos[:, b : b + 1]
                )
                t3 = spool.tile([128, SA - P], f32, tag="t3")
                nc.scalar.activation(t3[:], Yb[:, P:SA], Act.Relu, bias=cb)
                t4 = spool.tile([128, SA - P], f32, tag="t4")
                nc.scalar.activation(
                    t4[:], t3[:], Act.Square, accum_out=accRest[:, b : b + 1]
                )

            # transpose (3, 256) -> per-row (128, 2, 3) via PE with identity
            sbQ = mpool.tile([3, RPC], f32)
            nc.scalar.copy(sbQ[:], psumQ[:])
            psumT = ppool.tile([128, 2, 3], f32)
            for h in range(2):
                nc.tensor.matmul(
                    psumT[:, h, :],
                    sbQ[:, h * 128 : (h + 1) * 128],
                    eye3t[:],
                    start=True,
                    stop=True,
                )
            Kq = psumT[:, :, 0]
            S1q = psumT[:, :, 1]
            S2q = psumT[:, :, 2]

            # R_tail = (c*K + 2*S1)*c + S2
            w1 = mpool.tile([128, 2], f32)
            nc.vector.tensor_tensor(w1[:], c_cols[:], Kq, Alu.mult)
            w2 = mpool.tile([128, 2], f32)
            nc.vector.scalar_tensor_tensor(w2[:], S1q, 2.0, w1[:], Alu.mult, Alu.add)
            w3 = mpool.tile([128, 2], f32)
            nc.vector.tensor_tensor(w3[:], w2[:], c_cols[:], Alu.mult)
            Rt = mpool.tile([128, 2], f32)
            nc.vector.tensor_tensor(Rt[:], w3[:], S2q, Alu.add)

            Ra = mpool.tile([128, 2], f32)
            nc.vector.tensor_tensor(Ra[:], accPos[:], accRest[:], Alu.add)
            R = mpool.tile([128, 2], f32)
            nc.vector.tensor_tensor(R[:], Ra[:], Rt[:], Alu.add)

            # u updates, p, and the mean
            uas = mpool.tile([128, 2], f32)
            nc.vector.tensor_scalar_mul(uas[:], ua[:], 1.0 - GAMMA)
            ups = mpool.tile([128, 2], f32)
            nc.vector.tensor_scalar_mul(ups[:], up[:], 1.0 - GAMMA)
            uan = mpool.tile([128, 2], f32)
            nc.vector.scalar_tensor_tensor(uan[:], R[:], GAMMA / N, uas[:], Alu.mult, Alu.add)
            upn = mpool.tile([128, 2], f32)
            nc.vector.scalar_tensor_tensor(
                upn[:], accPos[:], GAMMA / N, ups[:], Alu.mult, Alu.add
            )
            inv = mpool.tile([128, 2], f32)
            nc.vector.reciprocal(inv[:], uan[:])
            g1 = mpool.tile([128, 2], f32)
            nc.vector.tensor_tensor(g1[:], upn[:], R[:], Alu.mult)
            g2 = mpool.tile([128, 2], f32)
            nc.vector.tensor_tensor(g2[:], g1[:], inv[:], Alu.mult)
            g3 = mpool.tile([128, 2], f32)
            nc.vector.tensor_tensor(g3[:], g2[:], accPos[:], Alu.subtract)
            g4 = mpool.tile([128, 2], f32)
            nc.vector.tensor_tensor(g4[:], g3[:], inv[:], Alu.mult)
            acc = mpool.tile([128, 1], f32)
            nc.vector.tensor_reduce(acc[:], g4[:], mybir.AxisListType.X, Alu.add)
            psumF = ppool.tile([1, 1], f32)
            nc.tensor.matmul(psumF[:], onest[:], acc[:], start=True, stop=True)
            outsb = mpool.tile([1, 1], f32)
            # fold the 1/(N*P) mean normalization into the final copy
            nc.scalar.mul(outsb[:], psumF[:], 1.0 / (float(N) * float(P)))
            nc.sync.dma_start(out[:], outsb[:])

    nc.compile()
    return nc


def get_nc():
    if "nc" not in _NC_CACHE:
        _NC_CACHE["nc"] = _build_nc()
    return _NC_CACHE["nc"]


def make_in_maps(y_pred, u_all, u_pos, index_s, n_pos):
    import ml_dtypes

    y = np.ascontiguousarray(np.asarray(y_pred, dtype=np.float32).reshape(N))
    u_all = np.asarray(u_all, dtype=np.float32).reshape(-1)
    u_pos = np.asarray(u_pos, dtype=np.float32).reshape(-1)
    idx = np.asarray(index_s).astype(np.int64).reshape(-1)[:P]
    ua_ps = u_all[idx]
    up_ps = u_pos[idx]
    f = y[:P]

    y_cols = np.ascontiguousarray(y.reshape(128, 128).T)  # [p, k] = y[k*128 + p]
    y_bf = y[:SA].astype(ml_dtypes.bfloat16)
    y_bc = np.ascontiguousarray(np.broadcast_to(y_bf[None, :], (128, SA)))
    eye3 = np.eye(3, dtype=np.float32)
    ones_col = np.ones((128, 1), dtype=np.float32)

    in_maps = []
    for c in range(NCORES):
        rows = slice(c * RPC, (c + 1) * RPC)
        f_bf = f[rows].astype(ml_dtypes.bfloat16)
        in_maps.append(
            {
                "y_bc": y_bc,
                "f_bc": np.ascontiguousarray(np.broadcast_to(f_bf[None, :], (128, RPC))),
                "y_cols": y_cols,
                "f_cols": np.ascontiguousarray(f[rows].reshape(2, 128).T),
                "ua_cols": np.ascontiguousarray(ua_ps[rows].reshape(2, 128).T),
                "up_cols": np.ascontiguousarray(up_ps[rows].reshape(2, 128).T),
                "eye3": eye3,
                "ones_col": ones_col,
            }
        )
    return in_maps


def kernel(**inputs):
    n_pos = int(np.asarray(inputs["n_pos"]))
    assert n_pos == P, f"kernel hardcodes n_pos={P}, got {n_pos}"
    in_maps = make_in_maps(
        inputs["y_pred"], inputs["u_all"], inputs["u_pos"], inputs["index_s"], n_pos
    )
    from concourse.bass_utils import run_bass_kernel_spmd

    nc = get_nc()
    res = run_bass_kernel_spmd(nc, in_maps, list(range(NCORES)))
    total = 0.0
    for r in res.results:
        total += float(r["out"][0, 0])
    return np.float32(total)


# revision 10
# speedup vs baseline: 1.1804x; 1.0106x over previous
"""Trainium2 Bass kernel for nn_APLoss (8 NeuronCores, SPMD row-sharded).

The reference loss collapses to per-row quantities: with c_i = 1 - y_pred[i],
s_ij = relu(c_i + y_pred[j])^2,  R_i = sum_j s_ij,  Rpos_i = sum_{j<2048} s_ij:

  u_a = 0.01*u_all[i] + 0.99*R_i/n        u_p = 0.01*u_pos[i] + 0.99*Rpos_i/n
  loss = mean_i[ (u_p * R_i/n)/u_a^2 - (Rpos_i/n)/u_a ]

Each core owns 256 of the 2048 positive rows (2 partition blocks), holds all of
y_pred, computes its partial sum of the per-row terms; the host sums the 8
partial scalars (the unshard step for a row-sharded scalar mean).

Per-core device pipeline:
  * ScalarE, j in [0, SA): T = Relu(y_j + c_i) with per-partition bias, then
    Square with accum_out -> row sums, split at j=2048 so Rpos falls out.
  * VectorE, j-chunks in [SA, 16384): B[j, r] = (f_r - (y_j+1) < 0) via one
    bf16 tensor_scalar per 128-j chunk (2 elem/cycle/lane).
  * TensorE: (K,S1,S2) = W_chunk^T @ B accumulated in PSUM over chunks (bf16
    in, fp32 acc), W = [1, y, y^2]; then R_tail = (c*K + 2*S1)*c + S2.
  * Small fp32 vector ops do the u-update / p / mean math; PE ones-matmul
    reduces over partitions; each core DMAs out one scalar.

Host-side prep is layout only: dtype casts, reshapes/transposes, the gather
u[index_s[:n_pos]], and replicating y/f slices across the 128 partitions.
"""

import numpy as np

try:
    import concourse.bass as bass  # noqa: F401
except ImportError:  # pragma: no cover
    import sys

    sys.path.insert(0, "/opt/trn_rl_repo")

N = 16384
P = 2048
NCORES = 8
RPC = P // NCORES  # 256 rows per core = 2 partition blocks
SA = 4096  # ScalarE covers j in [0, SA)
K0 = SA // 128  # first indicator j-chunk (32)
ND = (N - SA) // 128  # number of indicator j-chunks (96)
GAMMA = 0.99

_NC_CACHE = {}


def _build_nc():
    import concourse.tile as tile
    from concourse import bacc, mybir

    f32 = mybir.dt.float32
    bf16 = mybir.dt.bfloat16
    Alu = mybir.AluOpType
    Act = mybir.ActivationFunctionType

    nc = bacc.Bacc("TRN2", target_bir_lowering=False, debug=False, num_devices=NCORES)

    y_bc = nc.dram_tensor("y_bc", [128, SA], bf16, kind="ExternalInput").ap()
    f_bc = nc.dram_tensor("f_bc", [128, RPC], bf16, kind="ExternalInput").ap()
    y_cols = nc.dram_tensor("y_cols", [128, 128], f32, kind="ExternalInput").ap()
    f_cols = nc.dram_tensor("f_cols", [128, 2], f32, kind="ExternalInput").ap()
    ua_cols = nc.dram_tensor("ua_cols", [128, 2], f32, kind="ExternalInput").ap()
    up_cols = nc.dram_tensor("up_cols", [128, 2], f32, kind="ExternalInput").ap()
    eye3 = nc.dram_tensor("eye3", [3, 3], f32, kind="ExternalInput").ap()
    ones_col = nc.dram_tensor("ones_col", [128, 1], f32, kind="ExternalInput").ap()
    out = nc.dram_tensor("out", [1, 1], f32, kind="ExternalOutput").ap()

    with tile.TileContext(nc) as tc:
        with (
            tc.tile_pool(name="const", bufs=1) as cpool,
            tc.tile_pool(name="bpool", bufs=8) as bpool,
            tc.tile_pool(name="scratch", bufs=2) as spool,
            tc.tile_pool(name="small", bufs=1) as mpool,
            tc.tile_pool(name="psum", bufs=1, space="PSUM") as ppool,
        ):
            # DMA trigger instructions cost ~0.6us each on the issuing engine,
            # so spread them over three queues: the indicator-path gates
            # (f broadcast + y columns) first on sync, the y broadcast halves
            # on gpsimd/scalar (pos half first).
            Fb = cpool.tile([128, RPC], bf16)
            nc.sync.dma_start(Fb[:], f_bc[:])
            ycols = cpool.tile([128, 128], f32)
            nc.sync.dma_start(ycols[:], y_cols[:])
            Yb = cpool.tile([128, SA], bf16)
            nc.gpsimd.dma_start(Yb[:, 0:1024], y_bc[:, 0:1024])
            nc.scalar.dma_start(Yb[:, 1024:2048], y_bc[:, 1024:2048])
            nc.gpsimd.dma_start(Yb[:, 2048:3072], y_bc[:, 2048:3072])
            nc.scalar.dma_start(Yb[:, 3072:SA], y_bc[:, 3072:SA])
            fcols = cpool.tile([128, 2], f32)
            nc.sync.dma_start(fcols[:], f_cols[:])
            ua = cpool.tile([128, 2], f32)
            nc.sync.dma_start(ua[:], ua_cols[:])
            up = cpool.tile([128, 2], f32)
            nc.sync.dma_start(up[:], up_cols[:])
            eye3t = cpool.tile([3, 3], f32)
            nc.sync.dma_start(eye3t[:], eye3[:])
            onest = cpool.tile([128, 1], f32)
            nc.sync.dma_start(onest[:], ones_col[:])

            # y + 1 per-chunk bias columns; c = 1 - f per-block bias columns
            y1 = cpool.tile([128, 128], f32)
            nc.vector.tensor_scalar_add(y1[:], ycols[:], 1.0)
            c_cols = cpool.tile([128, 2], f32)
            nc.scalar.activation(c_cols[:], fcols[:], Act.Identity, bias=1.0, scale=-1.0)

            # W[:, t, :] = [1, y, y^2] in bf16 for chunk k = K0 + t
            W = cpool.tile([128, ND, 3], bf16)
            nc.scalar.activation(W[:, :, 0], ycols[:, K0:128], Act.Copy, bias=1.0, scale=0.0)
            nc.vector.tensor_copy(W[:, :, 1], ycols[:, K0:128])
            nc.scalar.activation(W[:, :, 2], ycols[:, K0:128], Act.Square)

            # --- VectorE + TensorE indicator path: j-chunks [K0, 128) ---
            psumQ = ppool.tile([3, RPC], f32)
            for t in range(ND):
                k = K0 + t
                Bt = bpool.tile([128, RPC], bf16, tag="bt")
                nc.vector.tensor_scalar(
                    Bt[:], Fb[:], y1[:, k : k + 1], 0.0, Alu.subtract, Alu.is_lt
                )
                nc.tensor.matmul(
                    psumQ[:], W[:, t, :], Bt[:], start=(t == 0), stop=(t == ND - 1)
                )

            # --- ScalarE path: j in [0, SA), split at the positive boundary ---
            accPos = mpool.tile([128, 2], f32)
            accRest = mpool.tile([128, 2], f32)
            for b in range(2):
                cb = c_cols[:, b : b + 1]
                t1 = spool.tile([128, P], f32, tag="t1")
                nc.scalar.activation(t1[:], Yb[:, 0:P], Act.Relu, bias=cb)
                t2 = spool.tile([128, P], f32, tag="t2")
                nc.scalar.activation(
                    t2[:], t1[:], Act.Square, accum_out=accPos[:, b : b + 1]
                )
                t3 = spool.tile([128, SA - P], f32, tag="t3")
                nc.scalar.activation(t3[:], Yb[:, P:SA], Act.Relu, bias=cb)
                t4 = spool.tile([128, SA - P], f32, tag="t4")
                nc.scalar.activation(
                    t4[:], t3[:], Act.Square, accum_out=accRest[:, b : b + 1]
                )

            # transpose (3, 256) -> per-row (128, 2, 3) via PE with identity
            sbQ = mpool.tile([3, RPC], f32)
            nc.scalar.copy(sbQ[:], psumQ[:])
            psumT = ppool.tile([128, 2, 3], f32)
            for h in range(2):
                nc.tensor.matmul(
                    psumT[:, h, :],
                    sbQ[:, h * 128 : (h + 1) * 128],
                    eye3t[:],
                    start=True,
                    stop=True,
                )
            Kq = psumT[:, :, 0]
            S1q = psumT[:, :, 1]
            S2q = psumT[:, :, 2]

            # R_tail = (c*K + 2*S1)*c + S2
            w1 = mpool.tile([128, 2], f32)
            nc.vector.tensor_tensor(w1[:], c_cols[:], Kq, Alu.mult)
            w2 = mpool.tile([128, 2], f32)
            nc.vector.scalar_tensor_tensor(w2[:], S1q, 2.0, w1[:], Alu.mult, Alu.add)
            w3 = mpool.tile([128, 2], f32)
            nc.vector.tensor_tensor(w3[:], w2[:], c_cols[:], Alu.mult)
            Rt = mpool.tile([128, 2], f32)
            nc.vector.tensor_tensor(Rt[:], w3[:], S2q, Alu.add)

            Ra = mpool.tile([128, 2], f32)
            nc.vector.tensor_tensor(Ra[:], accPos[:], accRest[:], Alu.add)
            R = mpool.tile([128, 2], f32)
            nc.vector.tensor_tensor(R[:], Ra[:], Rt[:], Alu.add)

            # u updates, p, and the mean
            uas = mpool.tile([128, 2], f32)
            nc.vector.tensor_scalar_mul(uas[:], ua[:], 1.0 - GAMMA)
            ups = mpool.tile([128, 2], f32)
            nc.vector.tensor_scalar_mul(ups[:], up[:], 1.0 - GAMMA)
            uan = mpool.tile([128, 2], f32)
            nc.vector.scalar_tensor_tensor(uan[:], R[:], GAMMA / N, uas[:], Alu.mult, Alu.add)
            upn = mpool.tile([128, 2], f32)
            nc.vector.scalar_tensor_tensor(
                upn[:], accPos[:], GAMMA / N, ups[:], Alu.mult, Alu.add
            )
            inv = mpool.tile([128, 2], f32)
            nc.vector.reciprocal(inv[:], uan[:])
            g1 = mpool.tile([128, 2], f32)
            nc.vector.tensor_tensor(g1[:], upn[:], R[:], Alu.mult)
            g2 = mpool.tile([128, 2], f32)
            nc.vector.tensor_tensor(g2[:], g1[:], inv[:], Alu.mult)
            g3 = mpool.tile([128, 2], f32)
            nc.vector.tensor_tensor(g3[:], g2[:], accPos[:], Alu.subtract)
            g4 = mpool.tile([128, 2], f32)
            nc.vector.tensor_tensor(g4[:], g3[:], inv[:], Alu.mult)
            acc = mpool.tile([128, 1], f32)
            nc.vector.tensor_reduce(acc[:], g4[:], mybir.AxisListType.X, Alu.add)
            psumF = ppool.tile([1, 1], f32)
            nc.tensor.matmul(psumF[:], onest[:], acc[:], start=True, stop=True)
            outsb = mpool.tile([1, 1], f32)
            # fold the 1/(N*P) mean normalization into the final copy
            nc.scalar.mul(outsb[:], psumF[:], 1.0 / (float(N) * float(P)))
            nc.sync.dma_start(out[:], outsb[:])

    nc.compile()
    return nc


def get_nc():
    if "nc" not in _NC_CACHE:
        _NC_CACHE["nc"] = _build_nc()
    return _NC_CACHE["nc"]


def make_in_maps(y_pred, u_all, u_pos, index_s, n_pos):
    import ml_dtypes

    y = np.ascontiguousarray(np.asarray(y_pred, dtype=np.float32).reshape(N))
    u_all = np.asarray(u_all, dtype=np.float32).reshape(-1)
    u_pos = np.asarray(u_pos, dtype=np.float32).reshape(-1)
    idx = np.asarray(index_s).astype(np.int64).reshape(-1)[:P]
    ua_ps = u_all[idx]
    up_ps = u_pos[idx]
    f = y[:P]

    y_cols = np.ascontiguousarray(y.reshape(128, 128).T)  # [p, k] = y[k*128 + p]
    y_bf = y[:SA].astype(ml_dtypes.bfloat16)
    y_bc = np.ascontiguousarray(np.broadcast_to(y_bf[None, :], (128, SA)))
    eye3 = np.eye(3, dtype=np.float32)
    ones_col = np.ones((128, 1), dtype=np.float32)

    in_maps = []
    for c in range(NCORES):
        rows = slice(c * RPC, (c + 1) * RPC)
        f_bf = f[rows].astype(ml_dtypes.bfloat16)
        in_maps.append(
            {
                "y_bc": y_bc,
                "f_bc": np.ascontiguousarray(np.broadcast_to(f_bf[None, :], (128, RPC))),
                "y_cols": y_cols,
                "f_cols": np.ascontiguousarray(f[rows].reshape(2, 128).T),
                "ua_cols": np.ascontiguousarray(ua_ps[rows].reshape(2, 128).T),
                "up_cols": np.ascontiguousarray(up_ps[rows].reshape(2, 128).T),
                "eye3": eye3,
                "ones_col": ones_col,
            }
        )
    return in_maps


def kernel(**inputs):
    n_pos = int(np.asarray(inputs["n_pos"]))
    assert n_pos == P, f"kernel hardcodes n_pos={P}, got {n_pos}"
    in_maps = make_in_maps(
        inputs["y_pred"], inputs["u_all"], inputs["u_pos"], inputs["index_s"], n_pos
    )
    from concourse.bass_utils import run_bass_kernel_spmd

    nc = get_nc()
    res = run_bass_kernel_spmd(nc, in_maps, list(range(NCORES)))
    total = 0.0
    for r in res.results:
        total += float(r["out"][0, 0])
    return np.float32(total)


# revision 16
# speedup vs baseline: 1.2016x; 1.0180x over previous
"""Trainium2 Bass kernel for nn_APLoss (8 NeuronCores, SPMD row-sharded).

The reference loss collapses to per-row quantities: with c_i = 1 - y_pred[i],
s_ij = relu(c_i + y_pred[j])^2,  R_i = sum_j s_ij,  Rpos_i = sum_{j<2048} s_ij:

  u_a = 0.01*u_all[i] + 0.99*R_i/n        u_p = 0.01*u_pos[i] + 0.99*Rpos_i/n
  loss = mean_i[ (u_p * R_i/n)/u_a^2 - (Rpos_i/n)/u_a ]

Each core owns 256 of the 2048 positive rows (2 partition blocks), holds all of
y_pred, computes its partial sum of the per-row terms; the host sums the 8
partial scalars (the unshard step for a row-sharded scalar mean).

Per-core device pipeline:
  * ScalarE, j in [0, SA): T = Relu(y_j + c_i) with per-partition bias, then
    Square with accum_out -> row sums, split at j=2048 so Rpos falls out.
  * VectorE, j-chunks in [SA, 16384): B[j, r] = (f_r - (y_j+1) < 0) via one
    bf16 tensor_scalar per 128-j chunk (2 elem/cycle/lane).
  * TensorE: (K,S1,S2) = W_chunk^T @ B accumulated in PSUM over chunks (bf16
    in, fp32 acc), W = [1, y, y^2]; then R_tail = (c*K + 2*S1)*c + S2.
  * Small fp32 vector ops do the u-update / p / mean math; PE ones-matmul
    reduces over partitions; each core DMAs out one scalar.

Host-side prep is layout only: dtype casts, reshapes/transposes, the gather
u[index_s[:n_pos]], and replicating y/f slices across the 128 partitions.
"""

import numpy as np

try:
    import concourse.bass as bass  # noqa: F401
except ImportError:  # pragma: no cover
    import sys

    sys.path.insert(0, "/opt/trn_rl_repo")

N = 16384
P = 2048
NCORES = 8
RPC = P // NCORES  # 256 rows per core = 2 partition blocks
SA = 4096  # ScalarE covers j in [0, SA)
K0 = SA // 128  # first indicator j-chunk (32)
ND = (N - SA) // 128  # number of indicator j-chunks (96)
GAMMA = 0.99

_NC_CACHE = {}


def _build_nc():
    import concourse.tile as tile
    from concourse import bacc, mybir

    f32 = mybir.dt.float32
    bf16 = mybir.dt.bfloat16
    Alu = mybir.AluOpType
    Act = mybir.ActivationFunctionType

    nc = bacc.Bacc("TRN2", target_bir_lowering=False, debug=False, num_devices=NCORES)

    y_bc = nc.dram_tensor("y_bc", [128, SA], bf16, kind="ExternalInput").ap()
    f_bc = nc.dram_tensor("f_bc", [128, RPC], bf16, kind="ExternalInput").ap()
    y_cols = nc.dram_tensor("y_cols", [128, 128], f32, kind="ExternalInput").ap()
    f_cols = nc.dram_tensor("f_cols", [128, 2], f32, kind="ExternalInput").ap()
    ua_cols = nc.dram_tensor("ua_cols", [128, 2], f32, kind="ExternalInput").ap()
    up_cols = nc.dram_tensor("up_cols", [128, 2], f32, kind="ExternalInput").ap()
    eye3 = nc.dram_tensor("eye3", [3, 3], f32, kind="ExternalInput").ap()
    ones_col = nc.dram_tensor("ones_col", [128, 1], f32, kind="ExternalInput").ap()
    out = nc.dram_tensor("out", [1, 1], f32, kind="ExternalOutput").ap()

    with tile.TileContext(nc) as tc:
        with (
            tc.tile_pool(name="const", bufs=1) as cpool,
            tc.tile_pool(name="bpool", bufs=8) as bpool,
            tc.tile_pool(name="scratch", bufs=2) as spool,
            tc.tile_pool(name="small", bufs=1) as mpool,
            tc.tile_pool(name="psum", bufs=1, space="PSUM") as ppool,
        ):
            # DMA trigger instructions cost ~0.6us each on the issuing engine,
            # so spread them over three queues: the indicator-path gates
            # (f broadcast + y columns) first on sync, the y broadcast halves
            # on gpsimd/scalar (pos half first).
            Fb = cpool.tile([128, RPC], bf16)
            nc.sync.dma_start(Fb[:], f_bc[:])
            ycols = cpool.tile([128, 128], f32)
            nc.sync.dma_start(ycols[:], y_cols[:])
            # tiny consumer ops so the big y broadcast DMAs are issued only
            # after the small gating transfers finish (the SDMA engines
            # round-robin all queued work, which would starve Fb/ycols).
            gate1 = cpool.tile([1, 1], bf16)
            nc.gpsimd.tensor_copy(gate1[:], Fb[0:1, 0:1])
            gate2 = cpool.tile([1, 1], f32)
            nc.scalar.copy(gate2[:], ycols[0:1, 0:1])
            Yb = cpool.tile([128, SA], bf16)
            nc.gpsimd.dma_start(Yb[:, 0:1024], y_bc[:, 0:1024])
            nc.scalar.dma_start(Yb[:, 1024:2048], y_bc[:, 1024:2048])
            nc.gpsimd.dma_start(Yb[:, 2048:3072], y_bc[:, 2048:3072])
            nc.scalar.dma_start(Yb[:, 3072:SA], y_bc[:, 3072:SA])
            fcols = cpool.tile([128, 2], f32)
            nc.sync.dma_start(fcols[:], f_cols[:])
            ua = cpool.tile([128, 2], f32)
            nc.sync.dma_start(ua[:], ua_cols[:])
            up = cpool.tile([128, 2], f32)
            nc.sync.dma_start(up[:], up_cols[:])
            eye3t = cpool.tile([3, 3], f32)
            nc.sync.dma_start(eye3t[:], eye3[:])
            onest = cpool.tile([128, 1], f32)
            nc.sync.dma_start(onest[:], ones_col[:])

            # y + 1 per-chunk bias columns; c = 1 - f per-block bias columns
            y1 = cpool.tile([128, 128], f32)
            nc.vector.tensor_scalar_add(y1[:], ycols[:], 1.0)
            c_cols = cpool.tile([128, 2], f32)
            nc.scalar.activation(c_cols[:], fcols[:], Act.Identity, bias=1.0, scale=-1.0)

            # W[:, t, :] = [1, y, y^2] in bf16 for chunk k = K0 + t
            W = cpool.tile([128, ND, 3], bf16)
            nc.scalar.activation(W[:, :, 0], ycols[:, K0:128], Act.Copy, bias=1.0, scale=0.0)
            nc.vector.tensor_copy(W[:, :, 1], ycols[:, K0:128])
            nc.scalar.activation(W[:, :, 2], ycols[:, K0:128], Act.Square)

            # --- VectorE + TensorE indicator path: j-chunks [K0, 128) ---
            psumQ = ppool.tile([3, RPC], f32)
            for t in range(ND):
                k = K0 + t
                Bt = bpool.tile([128, RPC], bf16, tag="bt")
                nc.vector.tensor_scalar(
                    Bt[:], Fb[:], y1[:, k : k + 1], 0.0, Alu.subtract, Alu.is_lt
                )
                nc.tensor.matmul(
                    psumQ[:], W[:, t, :], Bt[:], start=(t == 0), stop=(t == ND - 1)
                )

            # --- ScalarE path: j in [0, SA), split at the positive boundary ---
            accPos = mpool.tile([128, 2], f32)
            accRest = mpool.tile([128, 2], f32)
            for b in range(2):
                cb = c_cols[:, b : b + 1]
                t1 = spool.tile([128, P], f32, tag="t1")
                nc.scalar.activation(t1[:], Yb[:, 0:P], Act.Relu, bias=cb)
                t2 = spool.tile([128, P], f32, tag="t2")
                nc.scalar.activation(
                    t2[:], t1[:], Act.Square, accum_out=accPos[:, b : b + 1]
                )
                t3 = spool.tile([128, SA - P], f32, tag="t3")
                nc.scalar.activation(t3[:], Yb[:, P:SA], Act.Relu, bias=cb)
                t4 = spool.tile([128, SA - P], f32, tag="t4")
                nc.scalar.activation(
                    t4[:], t3[:], Act.Square, accum_out=accRest[:, b : b + 1]
                )

            # transpose (3, 256) -> per-row (128, 2, 3) via PE with identity;
            # group a is ready halfway through the chunk loop.
            sbQ = mpool.tile([3, RPC], f32)
            nc.scalar.copy(sbQ[:], psumQ[:])
            psumT = ppool.tile([128, 2, 3], f32)
            for h in range(2):
                nc.tensor.matmul(
                    psumT[:, h, :], sbQ[:, h * 128 : (h + 1) * 128], eye3t[:],
                    start=True, stop=True,
                )
            Kq = psumT[:, :, 0]
            S1q = psumT[:, :, 1]
            S2q = psumT[:, :, 2]

            # R_tail = (c*K + 2*S1)*c + S2
            w1 = mpool.tile([128, 2], f32)
            nc.vector.tensor_tensor(w1[:], c_cols[:], Kq, Alu.mult)
            w2 = mpool.tile([128, 2], f32)
            nc.vector.scalar_tensor_tensor(w2[:], S1q, 2.0, w1[:], Alu.mult, Alu.add)
            w3 = mpool.tile([128, 2], f32)
            nc.vector.tensor_tensor(w3[:], w2[:], c_cols[:], Alu.mult)
            Rt = mpool.tile([128, 2], f32)
            nc.vector.tensor_tensor(Rt[:], w3[:], S2q, Alu.add)

            Ra = mpool.tile([128, 2], f32)
            nc.vector.tensor_tensor(Ra[:], accPos[:], accRest[:], Alu.add)
            R = mpool.tile([128, 2], f32)
            nc.vector.tensor_tensor(R[:], Ra[:], Rt[:], Alu.add)

            # u updates, p, and the mean
            uas = mpool.tile([128, 2], f32)
            nc.vector.tensor_scalar_mul(uas[:], ua[:], 1.0 - GAMMA)
            ups = mpool.tile([128, 2], f32)
            nc.vector.tensor_scalar_mul(ups[:], up[:], 1.0 - GAMMA)
            uan = mpool.tile([128, 2], f32)
            nc.vector.scalar_tensor_tensor(uan[:], R[:], GAMMA / N, uas[:], Alu.mult, Alu.add)
            upn = mpool.tile([128, 2], f32)
            nc.vector.scalar_tensor_tensor(
                upn[:], accPos[:], GAMMA / N, ups[:], Alu.mult, Alu.add
            )
            inv = mpool.tile([128, 2], f32)
            nc.vector.reciprocal(inv[:], uan[:])
            g1 = mpool.tile([128, 2], f32)
            nc.vector.tensor_tensor(g1[:], upn[:], R[:], Alu.mult)
            g2 = mpool.tile([128, 2], f32)
            nc.vector.tensor_tensor(g2[:], g1[:], inv[:], Alu.mult)
            g3 = mpool.tile([128, 2], f32)
            nc.vector.tensor_tensor(g3[:], g2[:], accPos[:], Alu.subtract)
            g4 = mpool.tile([128, 2], f32)
            nc.vector.tensor_tensor(g4[:], g3[:], inv[:], Alu.mult)
            acc = mpool.tile([128, 1], f32)
            nc.vector.tensor_reduce(acc[:], g4[:], mybir.AxisListType.X, Alu.add)
            psumF = ppool.tile([1, 1], f32)
            nc.tensor.matmul(psumF[:], onest[:], acc[:], start=True, stop=True)
            outsb = mpool.tile([1, 1], f32)
            # fold the 1/(N*P) mean normalization into the final copy
            nc.scalar.mul(outsb[:], psumF[:], 1.0 / (float(N) * float(P)))
            nc.sync.dma_start(out[:], outsb[:])

    nc.compile()
    return nc


def get_nc():
    if "nc" not in _NC_CACHE:
        _NC_CACHE["nc"] = _build_nc()
    return _NC_CACHE["nc"]


def make_in_maps(y_pred, u_all, u_pos, index_s, n_pos):
    import ml_dtypes

    y = np.ascontiguousarray(np.asarray(y_pred, dtype=np.float32).reshape(N))
    u_all = np.asarray(u_all, dtype=np.float32).reshape(-1)
    u_pos = np.asarray(u_pos, dtype=np.float32).reshape(-1)
    idx = np.asarray(index_s).astype(np.int64).reshape(-1)[:P]
    ua_ps = u_all[idx]
    up_ps = u_pos[idx]
    f = y[:P]

    y_cols = np.ascontiguousarray(y.reshape(128, 128).T)  # [p, k] = y[k*128 + p]
    y_bf = y[:SA].astype(ml_dtypes.bfloat16)
    y_bc = np.ascontiguousarray(np.broadcast_to(y_bf[None, :], (128, SA)))
    eye3 = np.eye(3, dtype=np.float32)
    ones_col = np.ones((128, 1), dtype=np.float32)

    in_maps = []
    for c in range(NCORES):
        rows = slice(c * RPC, (c + 1) * RPC)
        f_bf = f[rows].astype(ml_dtypes.bfloat16)
        in_maps.append(
            {
                "y_bc": y_bc,
                "f_bc": np.ascontiguousarray(np.broadcast_to(f_bf[None, :], (128, RPC))),
                "y_cols": y_cols,
                "f_cols": np.ascontiguousarray(f[rows].reshape(2, 128).T),
                "ua_cols": np.ascontiguousarray(ua_ps[rows].reshape(2, 128).T),
                "up_cols": np.ascontiguousarray(up_ps[rows].reshape(2, 128).T),
                "eye3": eye3,
                "ones_col": ones_col,
            }
        )
    return in_maps


def kernel(**inputs):
    n_pos = int(np.asarray(inputs["n_pos"]))
    assert n_pos == P, f"kernel hardcodes n_pos={P}, got {n_pos}"
    in_maps = make_in_maps(
        inputs["y_pred"], inputs["u_all"], inputs["u_pos"], inputs["index_s"], n_pos
    )
    from concourse.bass_utils import run_bass_kernel_spmd

    nc = get_nc()
    res = run_bass_kernel_spmd(nc, in_maps, list(range(NCORES)))
    total = 0.0
    for r in res.results:
        total += float(r["out"][0, 0])
    return np.float32(total)
